# revision 15
# baseline (speedup 1.0000x reference)
"""CoevolExtractor fused kernel v3 for 8x trn2 NeuronCores (Bass/Tile).

Computation (reference):
    pair[b,i,l,j,m] = sum_n x_down[b,n,i,j] * x_down_w[b,n,l,m]
    pair = LayerNorm_{(j,m)}(pair) * a_2 + b_2        (eps=1e-5, biased var)
    out  = pair @ W + b                               # (1, L, L, 128)

Strategy: shard i (first residue axis) across 8 cores (24 i's each).

All heavy compute is fp8e4m3 DoubleRow matmuls (cost: out_cols * 0.5 cyc):
  - phase A: host splits A/8 and B into fp8 hi+lo; pp = pair/8 accumulates
    as AhBh+AhBl+AlBh (AlBl dropped, ~3e-4) -- 3 DR matmuls per 512-col
    psum region, k-tiles = the two n-halves.
  - pair hi/lo: ACT/DVE copy + DVE subtract from psum (fp8; the pair/8
    scaling keeps sq = hi*hi inside fp8 range as a plain multiply).
  - Linear: out = ph*Wh + ph*Wl + pl*Wh (weights x512, split fp8 hi+lo;
    scale folded into invstd).  24 units of (rt, g), 48 DR msteps each
    (16 main m-pairs + 32 corrections), K=32 band at partition 32g.
  - stats: sq = (pair/8)^2 fp8, summed over (j,m) by DR ones-matmuls into
    one psum accumulator; inv_eff = 1/sqrt(256*statb + 4096*eps)
    = 1/(64*sqrt(var+eps)) so out = psl * inv_eff + bconst exactly.
  - mean folded into W host-side; biased var from E[x^2] (mean^2 dropped).
  - cols are m-major (col = m*L + l).

hw quirks honored: gpsimd must not touch PSUM; DVE reads at most one PSUM
operand; tensor_tensor_reduce and DR matmuls with M=24 fault the exec unit
(M padded to 32); one matmul accumulation group keeps a single
tile_position and only its first matmul sets start (2KB zero-region).
"""

import os
from contextlib import ExitStack

import ml_dtypes
import numpy as np

import concourse.bass as bass
import concourse.tile as tile
from concourse import bacc, mybir
from concourse.bass_utils import run_bass_kernel_spmd

F32 = mybir.dt.float32
BF16 = mybir.dt.bfloat16
FP8 = mybir.dt.float8e4
PM = mybir.MatmulPerfMode
ACTF = mybir.ActivationFunctionType
ALU = mybir.AluOpType

B, N, L, J = 1, 256, 192, 32
D2 = J * J          # 1024
F = 128             # n_feat_out
NCORES = 8
LI = L // NCORES    # 24 i's per core
NRT = LI * J // 128  # 6 row tiles of (i4, j)
CC = 1024           # pair psum chunk cols
NCC = J * L // CC   # 6 chunks per row tile
NU = 4 * NRT        # 24 linear units (rt, g)
NMS = 48            # linear msteps per unit (16 main m-pairs + 32 corr)
WSC = 512.0         # weight pre-scale (64 * 8 for the A/8 input scale)
ASC = 0.125         # A inputs pre-scaled by 1/8: psum pp = pair/8
EPS = 1e-5
FP8NP = ml_dtypes.float8_e4m3


def build_kernel(ctx: ExitStack, tc: tile.TileContext, t):
    nc = tc.nc

    const = ctx.enter_context(tc.tile_pool(name="const", bufs=1))
    bpool = ctx.enter_context(tc.tile_pool(name="bp", bufs=1))
    pqpool = ctx.enter_context(tc.tile_pool(name="pq", bufs=3))
    sqpool = ctx.enter_context(tc.tile_pool(name="sqp", bufs=3))
    opool = ctx.enter_context(tc.tile_pool(name="opool", bufs=24))
    fpool = ctx.enter_context(tc.tile_pool(name="fpool", bufs=4))
    ipool = ctx.enter_context(tc.tile_pool(name="ipool", bufs=6))
    work = ctx.enter_context(tc.tile_pool(name="work", bufs=1))
    ppb = ctx.enter_context(tc.tile_pool(name="ppb", bufs=2, space="PSUM"))
    pslb = ctx.enter_context(tc.tile_pool(name="pslb", bufs=3, space="PSUM"))
    statp = ctx.enter_context(tc.tile_pool(name="statp", bufs=1, space="PSUM"))

    # ---- input DMAs in consumption order ----
    ah = const.tile([128, 2, LI * J], FP8, tag="ah")
    al = const.tile([128, 2, LI * J], FP8, tag="al")
    nc.sync.dma_start(ah[:], t["ah"][:])
    nc.sync.dma_start(al[:], t["al"][:])
    bh = bpool.tile([128, 2, J * L], FP8, tag="bh")
    bl = bpool.tile([128, 2, J * L], FP8, tag="bl")
    bcuts = [0, 1024, 3584, J * L]
    for c in range(3):
        sl = slice(bcuts[c], bcuts[c + 1])
        nc.sync.dma_start(bh[:, :, sl], t["bh"][:, :, sl])
        nc.sync.dma_start(bl[:, :, sl], t["bl"][:, :, sl])
    wmain = const.tile([128, J * F], FP8, tag="wmain")
    nc.sync.dma_start(wmain[:], t["wmain"][:])
    wcorr = const.tile([128, J * 2 * F], FP8, tag="wcorr")
    nc.sync.dma_start(wcorr[:], t["wcorr"][:])
    bones = const.tile([128, 2, NRT * 32], FP8, tag="bones")
    nc.sync.dma_start(bones[:], t["bones"][:])
    bcol_t = const.tile([128, 1], F32, tag="bcol")
    nc.sync.dma_start(bcol_t[:], t["bcol"][:])

    eps24 = work.tile([LI, 1], F32, tag="eps24")
    # inv_eff = 1/sqrt(256*statb + 4096*EPS) = 1/(64*sqrt(var+eps))
    nc.gpsimd.memset(eps24[:], 4096.0 * EPS)
    # preload the sqrt_and_others ACT table (serves Copy+Square+Sqrt)
    actwarm = work.tile([1, 2], F32, tag="actwarm")
    nc.gpsimd.memset(actwarm[:], 1.0)
    nc.scalar.activation(actwarm[0:1, 0:1], actwarm[0:1, 1:2], ACTF.Sqrt)
    stage_inv = work.tile([1, LI * L], F32, tag="stage_inv")

    # pair hi/lo, fp8, per row tile: [(i4,j32), (hl, m, l)]
    pairq = [None] * NRT
    sq_t = [None] * NRT
    psl_u = {}
    out_sb = {}

    # DR needs M % 32 == 0: 32-row ssq accumulator, rows LI..31 stay 0
    statb_full = statp.tile([32, 512], F32, tag="statb", name="statb")
    statb = statb_full[:, 0:L]

    wmv = wmain[:].rearrange("p (m f) -> p m f", m=J)
    wcv = wcorr[:].rearrange("p (m hl f) -> p m hl f", m=J, hl=2)

    # ---- linear msteps: fp8 DR, N=192, unit (rt, g) ----
    fill = {"u": 0, "s": 0}

    def _mstep(u, s):
        rt, g = u // 4, u % 4
        if s == 0:
            pslt = pslb.tile([128, 512], F32, tag="psl", name=f"psl{u}")
            psl_u[u] = pslt[:, 0:L]
        psl = psl_u[u]
        pqv = pairq[rt][:].rearrange("p hl (m l) -> p hl m l", m=J)
        gs = slice(32 * g, 32 * (g + 1))
        if s < 16:
            lhsT = wmv[gs, 2 * s:2 * s + 2, :]
            rhs = pqv[gs, 0, 2 * s:2 * s + 2, :]
        else:
            m = s - 16
            lhsT = wcv[gs, m, :, :]
            rhs = pqv[gs, :, m, :]
        nc.tensor.matmul(psl, lhsT, rhs,
                         start=(s == 0), stop=(s == NMS - 1),
                         perf_mode=PM.DoubleRow, skip_group_check=True,
                         tile_position=(32 * g, 0))

    def emit_tail(u):
        # stage raw psl to sbuf (frees the psum bank); scaled later
        osb = opool.tile([128, L], F32, tag="osb", name=f"osb{u}")
        nc.vector.tensor_copy(osb[:], psl_u[u])
        out_sb[u] = osb

    def emit_msteps(nsteps, limit_u):
        while nsteps > 0 and fill["u"] < limit_u:
            u, s = fill["u"], fill["s"]
            take = min(nsteps, NMS - s)
            for ss in range(s, s + take):
                _mstep(u, ss)
            nsteps -= take
            if s + take == NMS:
                emit_tail(u)
                fill["u"], fill["s"] = u + 1, 0
            else:
                fill["s"] = s + take

    # ---- ssq: 16 DR ones-matmuls per rt into statb ----
    def emit_ssq(rt):
        sqv = sq_t[rt][:].rearrange("p (m l) -> p m l", m=J)
        for mp in range(J // 2):
            nc.tensor.matmul(statb, bones[:, :, rt * 32:(rt + 1) * 32],
                             sqv[:, 2 * mp:2 * mp + 2, :],
                             start=(rt == 0 and mp == 0),
                             stop=(rt == NRT - 1 and mp == J // 2 - 1),
                             perf_mode=PM.DoubleRow, skip_group_check=True)

    # ---- phase A + chased vector ops + interleaved linear fills ----
    for rt in range(NRT):
        pairq[rt] = pqpool.tile([128, 2, J * L], FP8, tag="pq", name=f"pq{rt}")
        sq_t[rt] = sqpool.tile([128, J * L], FP8, tag="sq", name=f"sq{rt}")
        for cc in range(NCC):
            k = rt * NCC + cc
            pp = ppb.tile([128, CC], F32, tag="pp")
            for h in range(2):
                sl = slice(cc * CC + h * 512, cc * CC + (h + 1) * 512)
                for i, (wa, xb) in enumerate(((ah, bh), (ah, bl), (al, bh))):
                    nc.tensor.matmul(
                        pp[:, h * 512:(h + 1) * 512],
                        wa[:, :, rt * 128:(rt + 1) * 128], xb[:, :, sl],
                        start=(i == 0), stop=(i == 2),
                        perf_mode=PM.DoubleRow, skip_group_check=True)
            csl = slice(cc * CC, (cc + 1) * CC)
            hi = pairq[rt][:, 0, csl]
            lo = pairq[rt][:, 1, csl]
            with nc.allow_low_precision(reason="fp8 pair hi/lo + scaled sq"):
                # hi/sq/lo from psum; balance ACT vs DVE.  pp = pair/8 so
                # sq = hi*hi stays inside fp8 range as a plain multiply.
                if k % 3 == 2:
                    nc.vector.tensor_copy(hi, pp[:])
                    nc.scalar.activation(sq_t[rt][:, csl], pp[:], ACTF.Square)
                else:
                    nc.scalar.activation(hi, pp[:], ACTF.Copy)
                    nc.vector.tensor_mul(sq_t[rt][:, csl], hi, hi)
                nc.vector.tensor_sub(lo, pp[:], hi)
            # interleave linear msteps for complete rts (lag the vector
            # frontier so in-order PE never head-of-line blocks)
            if rt >= 1 and (rt > 1 or cc >= 2):
                emit_msteps(40, 4 * rt)
        if rt >= 2:
            emit_ssq(rt - 2)

    emit_msteps(NMS * NU, NU)
    emit_ssq(NRT - 2)
    emit_ssq(NRT - 1)

    # ---- finalize invstd (scaled by 1/64 for the weight pre-scale) ----
    std24 = work.tile([LI, L], F32, tag="std24")
    nc.scalar.activation(std24[:], statb[0:LI, :], ACTF.Sqrt,
                         bias=eps24[:], scale=256.0)
    inv24 = work.tile([LI, L], F32, tag="inv24")
    nc.vector.reciprocal(inv24[:], std24[:])
    nc.sync.dma_start(stage_inv[0:1, :].rearrange("o (i l) -> o i l", i=LI),
                      inv24[:])

    # ---- epilogue: scale by invstd, add bconst, 4 merged DMAs out ----
    y4 = t["y"][:, :].rearrange("f (h g l) -> f h g l", g=4, l=L)
    for g in range(4):
        fin = fpool.tile([128, NRT, L], F32, tag="fin", name=f"fin{g}")
        for rt in range(NRT):
            u = rt * 4 + g
            i = 4 * rt + g
            inv_bc = ipool.tile([128, L], F32, tag="inv_bc")
            nc.gpsimd.partition_broadcast(
                inv_bc[:], stage_inv[0:1, i * L:(i + 1) * L])
            nc.vector.tensor_mul(fin[:, rt, :], out_sb[u][:], inv_bc[:])
        nc.gpsimd.tensor_scalar_add(
            fin[:].rearrange("f h l -> f (h l)"),
            fin[:].rearrange("f h l -> f (h l)"), bcol_t[:])
        nc.sync.dma_start(y4[:, :, g, :], fin[:])


def build_program():
    nc = bacc.Bacc("TRN2", target_bir_lowering=False, debug=False,
                   num_devices=NCORES)
    t = {}
    t["ah"] = nc.dram_tensor("ah", [128, 2, LI * J], FP8, kind="ExternalInput").ap()
    t["al"] = nc.dram_tensor("al", [128, 2, LI * J], FP8, kind="ExternalInput").ap()
    t["bh"] = nc.dram_tensor("bh", [128, 2, J * L], FP8, kind="ExternalInput").ap()
    t["bl"] = nc.dram_tensor("bl", [128, 2, J * L], FP8, kind="ExternalInput").ap()
    t["wmain"] = nc.dram_tensor("wmain", [128, J * F], FP8, kind="ExternalInput").ap()
    t["wcorr"] = nc.dram_tensor("wcorr", [128, J * 2 * F], FP8, kind="ExternalInput").ap()
    t["bones"] = nc.dram_tensor("bones", [128, 2, NRT * 32], FP8, kind="ExternalInput").ap()
    t["bcol"] = nc.dram_tensor("bcol", [128, 1], F32, kind="ExternalInput").ap()
    t["y"] = nc.dram_tensor("y", [F, LI * L], F32, kind="ExternalOutput").ap()

    reps = int(os.environ.get("COEVOL_REPS", "1"))
    with tile.TileContext(nc) as tc:
        for _ in range(reps):
            with ExitStack() as ctx:
                build_kernel(ctx, tc, t)
    nc.compile()
    return nc


def _fp8_split(x):
    hi = x.astype(FP8NP)
    lo = (x - hi.astype(np.float32)).astype(FP8NP)
    return hi, lo


def host_inputs(x_down, x_down_w, a_2, b_2, W, b):
    """Host-side prep: fp8 hi/lo splits, m-major B, prescaled split weights."""
    A2 = np.ascontiguousarray(x_down.reshape(N, L * J).astype(np.float32)) * ASC
    # m-major cols: col = m*L + l
    B2 = np.ascontiguousarray(
        x_down_w.reshape(N, L, J).transpose(0, 2, 1).reshape(N, J * L)
        .astype(np.float32))
    Bh, Bl = _fp8_split(B2)
    # k-interleave: [n, c] -> [n % 128, n // 128, c]
    bh = np.ascontiguousarray(Bh.reshape(2, 128, J * L).transpose(1, 0, 2))
    bl = np.ascontiguousarray(Bl.reshape(2, 128, J * L).transpose(1, 0, 2))

    Wp = a_2.astype(np.float64)[:, None] * W.astype(np.float64)
    s_row = Wp.sum(axis=0)
    # fold the -s[f]*mean[t] LayerNorm correction into the weights
    Wpp = (Wp - s_row[None, :] / D2) * WSC          # [(j*32+m), f], prescaled
    Wh = Wpp.astype(np.float32).astype(FP8NP)
    Wl = (Wpp.astype(np.float32) - Wh.astype(np.float32)).astype(FP8NP)
    Wjmf_h = Wh.reshape(J, J, F)   # [j, m, f]
    Wjmf_l = Wl.reshape(J, J, F)
    # wmain[32g+j, m*F+f] = Wh[j,m,f]; wcorr[.., m, (Wl, Wh), f]
    wmain = np.tile(Wjmf_h.reshape(J, J * F), (4, 1))
    wcorr = np.tile(
        np.stack([Wjmf_l, Wjmf_h], axis=2).reshape(J, J * 2 * F), (4, 1))
    wmain = np.ascontiguousarray(wmain)
    wcorr = np.ascontiguousarray(wcorr)

    bconst = b_2.astype(np.float64) @ W.astype(np.float64) + b.astype(np.float64)
    bcol = bconst.astype(np.float32).reshape(F, 1)
    # bones[(32g+j), kt, rt*32 + i'] = 1 where i' == 4*rt + g
    bones = np.zeros((128, 2, NRT * 32), dtype=FP8NP)
    for rt in range(NRT):
        for g in range(4):
            bones[32 * g:32 * (g + 1), :, rt * 32 + 4 * rt + g] = 1.0

    in_maps = []
    for c in range(NCORES):
        Ac = A2[:, c * LI * J:(c + 1) * LI * J]
        Ahc, Alc = _fp8_split(Ac)
        in_maps.append({
            "ah": np.ascontiguousarray(
                Ahc.reshape(2, 128, LI * J).transpose(1, 0, 2)),
            "al": np.ascontiguousarray(
                Alc.reshape(2, 128, LI * J).transpose(1, 0, 2)),
            "bh": bh,
            "bl": bl,
            "wmain": wmain,
            "wcorr": wcorr,
            "bones": bones,
            "bcol": bcol,
        })
    return in_maps


_NC_CACHE = {}


def _get_program():
    if "nc" not in _NC_CACHE:
        _NC_CACHE["nc"] = build_program()
    return _NC_CACHE["nc"]


def kernel(**inputs) -> np.ndarray:
    nc = _get_program()
    inputs = {k: np.asarray(v) for k, v in inputs.items()}
    in_maps = host_inputs(**inputs)
    trace = bool(int(os.environ.get("COEVOL_TRACE", "0")))
    res = run_bass_kernel_spmd(nc, in_maps, list(range(NCORES)), trace=trace)
    if trace:
        _NC_CACHE["last_result"] = res
    # per-core y is [F, LI*L]; unshard to (B, L, L, F)
    slabs = [res.results[c]["y"].reshape(F, LI, L).transpose(1, 2, 0)
             for c in range(NCORES)]
    return np.concatenate(slabs, axis=0).reshape(B, L, L, F)


# revision 16
# speedup vs baseline: 1.1631x; 1.1631x over previous
"""CoevolExtractor fused kernel v3 for 8x trn2 NeuronCores (Bass/Tile).

Computation (reference):
    pair[b,i,l,j,m] = sum_n x_down[b,n,i,j] * x_down_w[b,n,l,m]
    pair = LayerNorm_{(j,m)}(pair) * a_2 + b_2        (eps=1e-5, biased var)
    out  = pair @ W + b                               # (1, L, L, 128)

Strategy: shard i (first residue axis) across 8 cores (24 i's each).

All heavy compute is fp8e4m3 DoubleRow matmuls (cost: out_cols * 0.5 cyc):
  - phase A: host splits A/8 and B into fp8 hi+lo; pp = pair/8 accumulates
    as AhBh+AhBl+AlBh (AlBl dropped, ~3e-4) -- 3 DR matmuls per 512-col
    psum region, k-tiles = the two n-halves.
  - pair hi/lo: ACT/DVE copy + DVE subtract from psum (fp8; the pair/8
    scaling keeps sq = hi*hi inside fp8 range as a plain multiply).
  - Linear: out = ph*Wh + ph*Wl + pl*Wh (weights x512, split fp8 hi+lo;
    scale folded into invstd).  24 units of (rt, g), 48 DR msteps each
    (16 main m-pairs + 32 corrections), K=32 band at partition 32g.
  - stats: sq = (pair/8)^2 fp8, summed over (j,m) by DR ones-matmuls into
    one psum accumulator; inv_eff = 1/sqrt(256*statb + 4096*eps)
    = 1/(64*sqrt(var+eps)) so out = psl * inv_eff + bconst exactly.
  - mean folded into W host-side; biased var from E[x^2] (mean^2 dropped).
  - cols are m-major (col = m*L + l).

hw quirks honored: gpsimd must not touch PSUM; DVE reads at most one PSUM
operand; tensor_tensor_reduce and DR matmuls with M=24 fault the exec unit
(M padded to 32); one matmul accumulation group keeps a single
tile_position and only its first matmul sets start (2KB zero-region).
"""

import os
from contextlib import ExitStack

import ml_dtypes
import numpy as np

import concourse.bass as bass
import concourse.tile as tile
from concourse import bacc, mybir
from concourse.bass_utils import run_bass_kernel_spmd

F32 = mybir.dt.float32
BF16 = mybir.dt.bfloat16
FP8 = mybir.dt.float8e4
PM = mybir.MatmulPerfMode
ACTF = mybir.ActivationFunctionType
ALU = mybir.AluOpType

B, N, L, J = 1, 256, 192, 32
D2 = J * J          # 1024
F = 128             # n_feat_out
NCORES = 8
LI = L // NCORES    # 24 i's per core
NRT = LI * J // 128  # 6 row tiles of (i4, j)
CC = 1024           # pair psum chunk cols
NCC = J * L // CC   # 6 chunks per row tile
NU = 4 * NRT        # 24 linear units (rt, g)
NMS = 48            # linear msteps per unit (16 main m-pairs + 32 corr)
WSC = 512.0         # weight pre-scale (64 * 8 for the A/8 input scale)
ASC = 0.125         # A inputs pre-scaled by 1/8: psum pp = pair/8
EPS = 1e-5
FP8NP = ml_dtypes.float8_e4m3


def build_kernel(ctx: ExitStack, tc: tile.TileContext, t):
    nc = tc.nc

    const = ctx.enter_context(tc.tile_pool(name="const", bufs=1))
    bpool = ctx.enter_context(tc.tile_pool(name="bp", bufs=1))
    pqpool = ctx.enter_context(tc.tile_pool(name="pq", bufs=3))
    sqpool = ctx.enter_context(tc.tile_pool(name="sqp", bufs=3))
    opool = ctx.enter_context(tc.tile_pool(name="opool", bufs=24))
    fpool = ctx.enter_context(tc.tile_pool(name="fpool", bufs=4))
    ipool = ctx.enter_context(tc.tile_pool(name="ipool", bufs=6))
    work = ctx.enter_context(tc.tile_pool(name="work", bufs=1))
    ppb = ctx.enter_context(tc.tile_pool(name="ppb", bufs=2, space="PSUM"))
    pslb = ctx.enter_context(tc.tile_pool(name="pslb", bufs=3, space="PSUM"))
    statp = ctx.enter_context(tc.tile_pool(name="statp", bufs=1, space="PSUM"))

    # ---- input DMAs in consumption order ----
    ah = const.tile([128, 2, LI * J], FP8, tag="ah")
    al = const.tile([128, 2, LI * J], FP8, tag="al")
    nc.sync.dma_start(ah[:], t["ah"][:])
    nc.sync.dma_start(al[:], t["al"][:])
    bh = bpool.tile([128, 2, J * L], FP8, tag="bh")
    bl = bpool.tile([128, 2, J * L], FP8, tag="bl")
    bcuts = [0, 1024, 3584, J * L]
    for c in range(3):
        sl = slice(bcuts[c], bcuts[c + 1])
        nc.sync.dma_start(bh[:, :, sl], t["bh"][:, :, sl])
        nc.sync.dma_start(bl[:, :, sl], t["bl"][:, :, sl])
    wmain = const.tile([128, J * F], FP8, tag="wmain")
    nc.sync.dma_start(wmain[:], t["wmain"][:])
    wlow = const.tile([128, J * F], FP8, tag="wlow")
    nc.sync.dma_start(wlow[:], t["wlow"][:])
    bones = const.tile([128, 2, NRT * 32], FP8, tag="bones")
    nc.sync.dma_start(bones[:], t["bones"][:])
    bcol_t = const.tile([128, 1], F32, tag="bcol")
    nc.sync.dma_start(bcol_t[:], t["bcol"][:])

    eps24 = work.tile([LI, 1], F32, tag="eps24")
    # inv_eff = 1/sqrt(256*statb + 4096*EPS) = 1/(64*sqrt(var+eps))
    nc.gpsimd.memset(eps24[:], 4096.0 * EPS)
    # preload the sqrt_and_others ACT table (serves Copy+Square+Sqrt)
    actwarm = work.tile([1, 2], F32, tag="actwarm")
    nc.gpsimd.memset(actwarm[:], 1.0)
    nc.scalar.activation(actwarm[0:1, 0:1], actwarm[0:1, 1:2], ACTF.Sqrt)
    stage_inv = work.tile([1, LI * L], F32, tag="stage_inv")

    # pair hi/lo, fp8, per row tile, SEPARATE tiles so the ACT hi-writes
    # and DVE lo-writes never WAW-serialize at tile granularity
    pairh = [None] * NRT
    pairl = [None] * NRT
    sq_t = [None] * NRT
    psl_u = {}
    out_sb = {}

    # DR needs M % 32 == 0: 32-row ssq accumulator, rows LI..31 stay 0
    statb_full = statp.tile([32, 512], F32, tag="statb", name="statb")
    statb = statb_full[:, 0:L]

    wmv = wmain[:].rearrange("p (m f) -> p m f", m=J)
    wlv = wlow[:].rearrange("p (m f) -> p m f", m=J)

    # ---- linear msteps: fp8 DR, N=192, unit (rt, g) ----
    fill = {"u": 0, "s": 0}

    def _mstep(u, s):
        rt, g = u // 4, u % 4
        if s == 0:
            pslt = pslb.tile([128, 512], F32, tag="psl", name=f"psl{u}")
            psl_u[u] = pslt[:, 0:L]
        psl = psl_u[u]
        phv = pairh[rt][:].rearrange("p (m l) -> p m l", m=J)
        plv = pairl[rt][:].rearrange("p (m l) -> p m l", m=J)
        gs = slice(32 * g, 32 * (g + 1))
        mp = s % 16
        if s < 16:          # ph * Wh
            lhsT = wmv[gs, 2 * mp:2 * mp + 2, :]
            rhs = phv[gs, 2 * mp:2 * mp + 2, :]
        elif s < 32:        # ph * Wl
            lhsT = wlv[gs, 2 * mp:2 * mp + 2, :]
            rhs = phv[gs, 2 * mp:2 * mp + 2, :]
        else:               # pl * Wh
            lhsT = wmv[gs, 2 * mp:2 * mp + 2, :]
            rhs = plv[gs, 2 * mp:2 * mp + 2, :]
        nc.tensor.matmul(psl, lhsT, rhs,
                         start=(s == 0), stop=(s == NMS - 1),
                         perf_mode=PM.DoubleRow, skip_group_check=True,
                         tile_position=(32 * g, 0))

    def emit_tail(u):
        # stage raw psl to sbuf (frees the psum bank); scaled later.
        # ACT Copy keeps this off the DVE, which lo-subs saturate.
        osb = opool.tile([128, L], F32, tag="osb", name=f"osb{u}")
        nc.scalar.activation(osb[:], psl_u[u], ACTF.Copy)
        out_sb[u] = osb

    def emit_msteps(nsteps, limit_u):
        while nsteps > 0 and fill["u"] < limit_u:
            u, s = fill["u"], fill["s"]
            take = min(nsteps, NMS - s)
            for ss in range(s, s + take):
                _mstep(u, ss)
            nsteps -= take
            if s + take == NMS:
                emit_tail(u)
                fill["u"], fill["s"] = u + 1, 0
            else:
                fill["s"] = s + take

    # ---- ssq: 16 DR ones-matmuls per rt into statb ----
    def emit_ssq(rt):
        sqv = sq_t[rt][:].rearrange("p (m l) -> p m l", m=J)
        for mp in range(J // 2):
            nc.tensor.matmul(statb, bones[:, :, rt * 32:(rt + 1) * 32],
                             sqv[:, 2 * mp:2 * mp + 2, :],
                             start=(rt == 0 and mp == 0),
                             stop=(rt == NRT - 1 and mp == J // 2 - 1),
                             perf_mode=PM.DoubleRow, skip_group_check=True)

    # ---- phase A + chased vector ops + interleaved linear fills ----
    for rt in range(NRT):
        pairh[rt] = pqpool.tile([128, J * L], FP8, tag="pqh", name=f"pqh{rt}")
        pairl[rt] = pqpool.tile([128, J * L], FP8, tag="pql", name=f"pql{rt}")
        sq_t[rt] = sqpool.tile([128, J * L], FP8, tag="sq", name=f"sq{rt}")
        sq_on_act = rt < 4
        for cc in range(NCC):
            pp = ppb.tile([128, CC], F32, tag="pp")
            for h in range(2):
                sl = slice(cc * CC + h * 512, cc * CC + (h + 1) * 512)
                for i, (wa, xb) in enumerate(((ah, bh), (ah, bl), (al, bh))):
                    nc.tensor.matmul(
                        pp[:, h * 512:(h + 1) * 512],
                        wa[:, :, rt * 128:(rt + 1) * 128], xb[:, :, sl],
                        start=(i == 0), stop=(i == 2),
                        perf_mode=PM.DoubleRow, skip_group_check=True)
            csl = slice(cc * CC, (cc + 1) * CC)
            hi = pairh[rt][:, csl]
            lo = pairl[rt][:, csl]
            with nc.allow_low_precision(reason="fp8 pair hi/lo + scaled sq"):
                # hi always ACT; lo always DVE; sq per-rt engine so each
                # sbuf tile has a single writer engine (no cross-engine
                # WAW serialization).  pp = pair/8 keeps sq = hi*hi in
                # fp8 range as a plain multiply.
                nc.scalar.activation(hi, pp[:], ACTF.Copy)
                if sq_on_act:
                    nc.scalar.activation(sq_t[rt][:, csl], pp[:], ACTF.Square)
                else:
                    nc.vector.tensor_mul(sq_t[rt][:, csl], hi, hi)
                nc.vector.tensor_sub(lo, pp[:], hi)
            # interleave linear msteps for complete rts (lag the vector
            # frontier so in-order PE never head-of-line blocks)
            if rt >= 1 and (rt > 1 or cc >= 2):
                emit_msteps(40, 4 * rt)
        if rt >= 2:
            emit_ssq(rt - 2)

    emit_msteps(NMS * NU, NU)
    emit_ssq(NRT - 2)
    emit_ssq(NRT - 1)

    # ---- finalize invstd (scaled by 1/64 for the weight pre-scale) ----
    std24 = work.tile([LI, L], F32, tag="std24")
    nc.scalar.activation(std24[:], statb[0:LI, :], ACTF.Sqrt,
                         bias=eps24[:], scale=256.0)
    inv24 = work.tile([LI, L], F32, tag="inv24")
    nc.vector.reciprocal(inv24[:], std24[:])
    nc.sync.dma_start(stage_inv[0:1, :].rearrange("o (i l) -> o i l", i=LI),
                      inv24[:])

    # ---- epilogue: scale by invstd, add bconst, 4 merged DMAs out ----
    y4 = t["y"][:, :].rearrange("f (h g l) -> f h g l", g=4, l=L)
    for g in range(4):
        fin = fpool.tile([128, NRT, L], F32, tag="fin", name=f"fin{g}")
        for rt in range(NRT):
            u = rt * 4 + g
            i = 4 * rt + g
            inv_bc = ipool.tile([128, L], F32, tag="inv_bc")
            nc.gpsimd.partition_broadcast(
                inv_bc[:], stage_inv[0:1, i * L:(i + 1) * L])
            nc.vector.tensor_mul(fin[:, rt, :], out_sb[u][:], inv_bc[:])
        nc.gpsimd.tensor_scalar_add(
            fin[:].rearrange("f h l -> f (h l)"),
            fin[:].rearrange("f h l -> f (h l)"), bcol_t[:])
        nc.sync.dma_start(y4[:, :, g, :], fin[:])


def build_program():
    nc = bacc.Bacc("TRN2", target_bir_lowering=False, debug=False,
                   num_devices=NCORES)
    t = {}
    t["ah"] = nc.dram_tensor("ah", [128, 2, LI * J], FP8, kind="ExternalInput").ap()
    t["al"] = nc.dram_tensor("al", [128, 2, LI * J], FP8, kind="ExternalInput").ap()
    t["bh"] = nc.dram_tensor("bh", [128, 2, J * L], FP8, kind="ExternalInput").ap()
    t["bl"] = nc.dram_tensor("bl", [128, 2, J * L], FP8, kind="ExternalInput").ap()
    t["wmain"] = nc.dram_tensor("wmain", [128, J * F], FP8, kind="ExternalInput").ap()
    t["wlow"] = nc.dram_tensor("wlow", [128, J * F], FP8, kind="ExternalInput").ap()
    t["bones"] = nc.dram_tensor("bones", [128, 2, NRT * 32], FP8, kind="ExternalInput").ap()
    t["bcol"] = nc.dram_tensor("bcol", [128, 1], F32, kind="ExternalInput").ap()
    t["y"] = nc.dram_tensor("y", [F, LI * L], F32, kind="ExternalOutput").ap()

    reps = int(os.environ.get("COEVOL_REPS", "1"))
    with tile.TileContext(nc) as tc:
        for _ in range(reps):
            with ExitStack() as ctx:
                build_kernel(ctx, tc, t)
    nc.compile()
    return nc


def _fp8_split(x):
    hi = x.astype(FP8NP)
    lo = (x - hi.astype(np.float32)).astype(FP8NP)
    return hi, lo


def host_inputs(x_down, x_down_w, a_2, b_2, W, b):
    """Host-side prep: fp8 hi/lo splits, m-major B, prescaled split weights."""
    A2 = np.ascontiguousarray(x_down.reshape(N, L * J).astype(np.float32)) * ASC
    # m-major cols: col = m*L + l
    B2 = np.ascontiguousarray(
        x_down_w.reshape(N, L, J).transpose(0, 2, 1).reshape(N, J * L)
        .astype(np.float32))
    Bh, Bl = _fp8_split(B2)
    # k-interleave: [n, c] -> [n % 128, n // 128, c]
    bh = np.ascontiguousarray(Bh.reshape(2, 128, J * L).transpose(1, 0, 2))
    bl = np.ascontiguousarray(Bl.reshape(2, 128, J * L).transpose(1, 0, 2))

    Wp = a_2.astype(np.float64)[:, None] * W.astype(np.float64)
    s_row = Wp.sum(axis=0)
    # fold the -s[f]*mean[t] LayerNorm correction into the weights
    Wpp = (Wp - s_row[None, :] / D2) * WSC          # [(j*32+m), f], prescaled
    Wh = Wpp.astype(np.float32).astype(FP8NP)
    Wl = (Wpp.astype(np.float32) - Wh.astype(np.float32)).astype(FP8NP)
    Wjmf_h = Wh.reshape(J, J, F)   # [j, m, f]
    Wjmf_l = Wl.reshape(J, J, F)
    # wmain[32g+j, m*F+f] = Wh[j,m,f]; wlow likewise for Wl
    wmain = np.ascontiguousarray(np.tile(Wjmf_h.reshape(J, J * F), (4, 1)))
    wlow = np.ascontiguousarray(np.tile(Wjmf_l.reshape(J, J * F), (4, 1)))

    bconst = b_2.astype(np.float64) @ W.astype(np.float64) + b.astype(np.float64)
    bcol = bconst.astype(np.float32).reshape(F, 1)
    # bones[(32g+j), kt, rt*32 + i'] = 1 where i' == 4*rt + g
    bones = np.zeros((128, 2, NRT * 32), dtype=FP8NP)
    for rt in range(NRT):
        for g in range(4):
            bones[32 * g:32 * (g + 1), :, rt * 32 + 4 * rt + g] = 1.0

    in_maps = []
    for c in range(NCORES):
        Ac = A2[:, c * LI * J:(c + 1) * LI * J]
        Ahc, Alc = _fp8_split(Ac)
        in_maps.append({
            "ah": np.ascontiguousarray(
                Ahc.reshape(2, 128, LI * J).transpose(1, 0, 2)),
            "al": np.ascontiguousarray(
                Alc.reshape(2, 128, LI * J).transpose(1, 0, 2)),
            "bh": bh,
            "bl": bl,
            "wmain": wmain,
            "wlow": wlow,
            "bones": bones,
            "bcol": bcol,
        })
    return in_maps


_NC_CACHE = {}


def _get_program():
    if "nc" not in _NC_CACHE:
        _NC_CACHE["nc"] = build_program()
    return _NC_CACHE["nc"]


def kernel(**inputs) -> np.ndarray:
    nc = _get_program()
    inputs = {k: np.asarray(v) for k, v in inputs.items()}
    in_maps = host_inputs(**inputs)
    trace = bool(int(os.environ.get("COEVOL_TRACE", "0")))
    res = run_bass_kernel_spmd(nc, in_maps, list(range(NCORES)), trace=trace)
    if trace:
        _NC_CACHE["last_result"] = res
    # per-core y is [F, LI*L]; unshard to (B, L, L, F)
    slabs = [res.results[c]["y"].reshape(F, LI, L).transpose(1, 2, 0)
             for c in range(NCORES)]
    return np.concatenate(slabs, axis=0).reshape(B, L, L, F)


# revision 18
# speedup vs baseline: 1.1895x; 1.0227x over previous
"""CoevolExtractor fused kernel v3 for 8x trn2 NeuronCores (Bass/Tile).

Computation (reference):
    pair[b,i,l,j,m] = sum_n x_down[b,n,i,j] * x_down_w[b,n,l,m]
    pair = LayerNorm_{(j,m)}(pair) * a_2 + b_2        (eps=1e-5, biased var)
    out  = pair @ W + b                               # (1, L, L, 128)

Strategy: shard i (first residue axis) across 8 cores (24 i's each).

All heavy compute is fp8e4m3 DoubleRow matmuls (cost: out_cols * 0.5 cyc):
  - phase A: host splits A/8 and B into fp8 hi+lo; pp = pair/8 accumulates
    as AhBh+AhBl+AlBh (AlBl dropped, ~3e-4) -- 3 DR matmuls per 512-col
    psum region, k-tiles = the two n-halves.
  - pair hi/lo: ACT/DVE copy + DVE subtract from psum (fp8; the pair/8
    scaling keeps sq = hi*hi inside fp8 range as a plain multiply).
  - Linear: out = ph*Wh + ph*Wl + pl*Wh (weights x512, split fp8 hi+lo;
    scale folded into invstd).  24 units of (rt, g), 48 DR msteps each
    (16 main m-pairs + 32 corrections), K=32 band at partition 32g.
  - stats: sq = (pair/8)^2 fp8, summed over (j,m) by DR ones-matmuls into
    one psum accumulator; inv_eff = 1/sqrt(256*statb + 4096*eps)
    = 1/(64*sqrt(var+eps)) so out = psl * inv_eff + bconst exactly.
  - mean folded into W host-side; biased var from E[x^2] (mean^2 dropped).
  - cols are m-major (col = m*L + l).

hw quirks honored: gpsimd must not touch PSUM; DVE reads at most one PSUM
operand; tensor_tensor_reduce and DR matmuls with M=24 fault the exec unit
(M padded to 32); one matmul accumulation group keeps a single
tile_position and only its first matmul sets start (2KB zero-region).
"""

import os
from contextlib import ExitStack

import ml_dtypes
import numpy as np

import concourse.bass as bass
import concourse.tile as tile
from concourse import bacc, mybir
from concourse.bass_utils import run_bass_kernel_spmd

F32 = mybir.dt.float32
BF16 = mybir.dt.bfloat16
FP8 = mybir.dt.float8e4
PM = mybir.MatmulPerfMode
ACTF = mybir.ActivationFunctionType
ALU = mybir.AluOpType

B, N, L, J = 1, 256, 192, 32
D2 = J * J          # 1024
F = 128             # n_feat_out
NCORES = 8
LI = L // NCORES    # 24 i's per core
NRT = LI * J // 128  # 6 row tiles of (i4, j)
CC = 1024           # pair psum chunk cols
NCC = J * L // CC   # 6 chunks per row tile
NU = 4 * NRT        # 24 linear units (rt, g)
NMS = 48            # linear msteps per unit (16 main m-pairs + 32 corr)
WSC = 512.0         # weight pre-scale (64 * 8 for the A/8 input scale)
ASC = 0.125         # A inputs pre-scaled by 1/8: psum pp = pair/8
EPS = 1e-5
FP8NP = ml_dtypes.float8_e4m3


def build_kernel(ctx: ExitStack, tc: tile.TileContext, t):
    nc = tc.nc

    const = ctx.enter_context(tc.tile_pool(name="const", bufs=1))
    bpool = ctx.enter_context(tc.tile_pool(name="bp", bufs=1))
    pqpool = ctx.enter_context(tc.tile_pool(name="pq", bufs=3))
    sqpool = ctx.enter_context(tc.tile_pool(name="sqp", bufs=3))
    opool = ctx.enter_context(tc.tile_pool(name="opool", bufs=24))
    fpool = ctx.enter_context(tc.tile_pool(name="fpool", bufs=4))
    ipool = ctx.enter_context(tc.tile_pool(name="ipool", bufs=6))
    work = ctx.enter_context(tc.tile_pool(name="work", bufs=1))
    ppb = ctx.enter_context(tc.tile_pool(name="ppb", bufs=2, space="PSUM"))
    pslb = ctx.enter_context(tc.tile_pool(name="pslb", bufs=3, space="PSUM"))
    statp = ctx.enter_context(tc.tile_pool(name="statp", bufs=1, space="PSUM"))

    # ---- input DMAs in consumption order ----
    ah = const.tile([128, 2, LI * J], FP8, tag="ah")
    al = const.tile([128, 2, LI * J], FP8, tag="al")
    nc.sync.dma_start(ah[:], t["ah"][:])
    nc.sync.dma_start(al[:], t["al"][:])
    bh = bpool.tile([128, 2, J * L], FP8, tag="bh")
    bl = bpool.tile([128, 2, J * L], FP8, tag="bl")
    bcuts = [0, 1024, 3584, J * L]
    for c in range(3):
        sl = slice(bcuts[c], bcuts[c + 1])
        nc.sync.dma_start(bh[:, :, sl], t["bh"][:, :, sl])
        nc.sync.dma_start(bl[:, :, sl], t["bl"][:, :, sl])
    wmain = const.tile([128, J * F], FP8, tag="wmain")
    nc.sync.dma_start(wmain[:], t["wmain"][:])
    wlow = const.tile([128, J * F], FP8, tag="wlow")
    nc.sync.dma_start(wlow[:], t["wlow"][:])
    bones = const.tile([128, 2, NRT * 32], FP8, tag="bones")
    nc.sync.dma_start(bones[:], t["bones"][:])
    bcol_t = const.tile([128, 1], F32, tag="bcol")
    nc.sync.dma_start(bcol_t[:], t["bcol"][:])

    eps24 = work.tile([LI, 1], F32, tag="eps24")
    # inv_eff = 1/sqrt(256*statb + 4096*EPS) = 1/(64*sqrt(var+eps))
    nc.gpsimd.memset(eps24[:], 4096.0 * EPS)
    # preload the sqrt_and_others ACT table (serves Copy+Square+Sqrt)
    actwarm = work.tile([1, 2], F32, tag="actwarm")
    nc.gpsimd.memset(actwarm[:], 1.0)
    nc.scalar.activation(actwarm[0:1, 0:1], actwarm[0:1, 1:2], ACTF.Sqrt)
    stage_inv = work.tile([1, LI * L], F32, tag="stage_inv")

    # pair hi/lo, fp8, per row tile, SEPARATE tiles so the ACT hi-writes
    # and DVE lo-writes never WAW-serialize at tile granularity
    pairh = [None] * NRT
    pairl = [None] * NRT
    sq_t = [None] * NRT
    psl_u = {}
    out_sb = {}

    # DR needs M % 32 == 0: 32-row ssq accumulator, rows LI..31 stay 0
    statb_full = statp.tile([32, 512], F32, tag="statb", name="statb")
    statb = statb_full[:, 0:L]

    wmv = wmain[:].rearrange("p (m f) -> p m f", m=J)
    wlv = wlow[:].rearrange("p (m f) -> p m f", m=J)

    # ---- linear msteps: fp8 DR, N=192, unit (rt, g); step order is
    # m-pair-major (mp, term) so steps become emittable as soon as the
    # vector frontier covers their m-pair, enabling rt-local interleave ----
    nsteps_done = [0] * NU
    psl_of = {}

    def _mstep(u, s):
        rt, g = u // 4, u % 4
        if s == 0:
            pslt = pslb.tile([128, 512], F32, tag="psl", name=f"psl{u}")
            psl_of[u] = pslt[:, 0:L]
        psl = psl_of[u]
        phv = pairh[rt][:].rearrange("p (m l) -> p m l", m=J)
        plv = pairl[rt][:].rearrange("p (m l) -> p m l", m=J)
        gs = slice(32 * g, 32 * (g + 1))
        mp, term = s // 3, s % 3
        msl = slice(2 * mp, 2 * mp + 2)
        if term == 0:       # ph * Wh
            lhsT, rhs = wmv[gs, msl, :], phv[gs, msl, :]
        elif term == 1:     # ph * Wl
            lhsT, rhs = wlv[gs, msl, :], phv[gs, msl, :]
        else:               # pl * Wh
            lhsT, rhs = wmv[gs, msl, :], plv[gs, msl, :]
        nc.tensor.matmul(psl, lhsT, rhs,
                         start=(s == 0), stop=(s == NMS - 1),
                         perf_mode=PM.DoubleRow, skip_group_check=True,
                         tile_position=(32 * g, 0))

    ep_pending = []

    def emit_tail(u):
        # stage raw psl to sbuf on ACT (frees the psum bank); scaled later
        osb = opool.tile([128, L], F32, tag="osb", name=f"osb{u}")
        nc.scalar.activation(osb[:], psl_of[u], ACTF.Copy)
        out_sb[u] = osb
        ep_pending.append(u)

    def unit_gate(u, rt, cc):
        # max emittable step count for unit u after chunk (rt, cc)
        urt = u // 4
        if urt < rt:
            return NMS
        if urt > rt:
            return 0
        covered_m = cc * CC // L          # full m's covered by chunks < cc
        mp_lim = (covered_m - 1) // 2     # m-pairs fully covered
        return max(0, min(NMS, 3 * mp_lim))

    def emit_fill(budget, rt, cc):
        # round-robin over a window of <=3 incomplete units (3 psum banks)
        while budget > 0:
            window = [u for u in range(NU) if nsteps_done[u] < NMS][:3]
            progress = False
            for u in window:
                gate = unit_gate(u, rt, cc)
                take = min(budget, gate - nsteps_done[u], 12)
                if take <= 0:
                    continue
                s0 = nsteps_done[u]
                for s in range(s0, s0 + take):
                    _mstep(u, s)
                nsteps_done[u] = s0 + take
                budget -= take
                progress = True
                if nsteps_done[u] == NMS:
                    emit_tail(u)
            if not progress:
                return

    # ---- ssq: 16 DR ones-matmuls per rt into statb ----
    def emit_ssq(rt):
        sqv = sq_t[rt][:].rearrange("p (m l) -> p m l", m=J)
        for mp in range(J // 2):
            nc.tensor.matmul(statb, bones[:, :, rt * 32:(rt + 1) * 32],
                             sqv[:, 2 * mp:2 * mp + 2, :],
                             start=(rt == 0 and mp == 0),
                             stop=(rt == NRT - 1 and mp == J // 2 - 1),
                             perf_mode=PM.DoubleRow, skip_group_check=True)

    # ---- per-rt finalize: rt's statb rows [4rt, 4rt+4) are final once
    # ssq(rt) is emitted (program order gives the dependency slice) ----
    slab_done = [False] * NRT

    def emit_slab(rt):
        # engines may not address partition bases like 4, so compute the
        # whole [0:LI] each time (rows > 4rt+3 are still accumulating and
        # produce garbage, but only this rt's finished 4-row slab is
        # staged out via DMA, which has no partition-base restriction)
        csl = slice(4 * rt * L, (4 * rt + 4) * L)
        stds = work.tile([LI, L], F32, tag=f"stds{rt}", name=f"stds{rt}")
        nc.scalar.activation(stds[:], statb[0:LI, :], ACTF.Sqrt,
                             bias=eps24[:], scale=256.0)
        invs = work.tile([LI, L], F32, tag=f"invs{rt}", name=f"invs{rt}")
        nc.vector.reciprocal(invs[:], stds[:])
        nc.sync.dma_start(
            stage_inv[0:1, csl].rearrange("o (i l) -> o i l", i=4),
            invs[4 * rt:4 * rt + 4, :])
        slab_done[rt] = True

    # ---- epilogue stream: per finished unit, scale by its invstd row ----
    fin = [None] * 4
    fin_parts = [0] * 4
    y4 = t["y"][:, :].rearrange("f (h g l) -> f h g l", g=4, l=L)

    def pump_epilogue():
        done = []
        for u in ep_pending:
            rt, g = u // 4, u % 4
            if not slab_done[rt]:
                continue
            i = 4 * rt + g
            if fin[g] is None:
                fin[g] = fpool.tile([128, NRT, L], F32, tag="fin",
                                    name=f"fin{g}")
            inv_bc = ipool.tile([128, L], F32, tag="inv_bc")
            nc.gpsimd.partition_broadcast(
                inv_bc[:], stage_inv[0:1, i * L:(i + 1) * L])
            nc.vector.tensor_mul(fin[g][:, rt, :], out_sb[u][:], inv_bc[:])
            done.append(u)
            fin_parts[g] += 1
            for half in (0, 1):
                # flush a g-half as soon as its 3 rt-slabs are scaled
                need = {half * 3, half * 3 + 1, half * 3 + 2}
                have = {uu // 4 for uu in range(g, NU, 4)
                        if uu in done or uu not in ep_pending}
        for u in done:
            ep_pending.remove(u)

    def flush_outputs():
        for g in range(4):
            for half in (0, 1):
                hs = slice(3 * half, 3 * half + 3)
                nc.gpsimd.tensor_scalar_add(
                    fin[g][:, hs, :].rearrange("f h l -> f (h l)"),
                    fin[g][:, hs, :].rearrange("f h l -> f (h l)"),
                    bcol_t[:])
                nc.sync.dma_start(y4[:, hs, g, :], fin[g][:, hs, :])

    # ---- phase A + chased vector ops + interleaved fills ----
    for rt in range(NRT):
        pairh[rt] = pqpool.tile([128, J * L], FP8, tag="pqh", name=f"pqh{rt}")
        pairl[rt] = pqpool.tile([128, J * L], FP8, tag="pql", name=f"pql{rt}")
        sq_t[rt] = sqpool.tile([128, J * L], FP8, tag="sq", name=f"sq{rt}")
        sq_on_act = rt % 3 != 2
        for cc in range(NCC):
            pp = ppb.tile([128, CC], F32, tag="pp")
            for h in range(2):
                sl = slice(cc * CC + h * 512, cc * CC + (h + 1) * 512)
                for i, (wa, xb) in enumerate(((ah, bh), (ah, bl), (al, bh))):
                    nc.tensor.matmul(
                        pp[:, h * 512:(h + 1) * 512],
                        wa[:, :, rt * 128:(rt + 1) * 128], xb[:, :, sl],
                        start=(i == 0), stop=(i == 2),
                        perf_mode=PM.DoubleRow, skip_group_check=True)
            csl = slice(cc * CC, (cc + 1) * CC)
            hi = pairh[rt][:, csl]
            lo = pairl[rt][:, csl]
            with nc.allow_low_precision(reason="fp8 pair hi/lo + scaled sq"):
                # hi always ACT; lo always DVE; sq per-rt engine so every
                # sbuf tile has one writer engine (no cross-engine WAW
                # serialization).  pp = pair/8 keeps sq = hi*hi in fp8.
                nc.scalar.activation(hi, pp[:], ACTF.Copy)
                if sq_on_act:
                    nc.scalar.activation(sq_t[rt][:, csl], pp[:], ACTF.Square)
                else:
                    nc.vector.tensor_mul(sq_t[rt][:, csl], hi, hi)
                nc.vector.tensor_sub(lo, pp[:], hi)
            if rt >= 1 and cc == 2:
                emit_ssq(rt - 1)
                emit_slab(rt - 1)
            emit_fill(40, rt, cc)
            pump_epilogue()

    emit_fill(NMS * NU, NRT, 0)
    emit_ssq(NRT - 1)
    emit_slab(NRT - 1)
    pump_epilogue()
    flush_outputs()


def build_program():
    nc = bacc.Bacc("TRN2", target_bir_lowering=False, debug=False,
                   num_devices=NCORES)
    t = {}
    t["ah"] = nc.dram_tensor("ah", [128, 2, LI * J], FP8, kind="ExternalInput").ap()
    t["al"] = nc.dram_tensor("al", [128, 2, LI * J], FP8, kind="ExternalInput").ap()
    t["bh"] = nc.dram_tensor("bh", [128, 2, J * L], FP8, kind="ExternalInput").ap()
    t["bl"] = nc.dram_tensor("bl", [128, 2, J * L], FP8, kind="ExternalInput").ap()
    t["wmain"] = nc.dram_tensor("wmain", [128, J * F], FP8, kind="ExternalInput").ap()
    t["wlow"] = nc.dram_tensor("wlow", [128, J * F], FP8, kind="ExternalInput").ap()
    t["bones"] = nc.dram_tensor("bones", [128, 2, NRT * 32], FP8, kind="ExternalInput").ap()
    t["bcol"] = nc.dram_tensor("bcol", [128, 1], F32, kind="ExternalInput").ap()
    t["y"] = nc.dram_tensor("y", [F, LI * L], F32, kind="ExternalOutput").ap()

    reps = int(os.environ.get("COEVOL_REPS", "1"))
    with tile.TileContext(nc) as tc:
        for _ in range(reps):
            with ExitStack() as ctx:
                build_kernel(ctx, tc, t)
    nc.compile()
    return nc


def _fp8_split(x):
    hi = x.astype(FP8NP)
    lo = (x - hi.astype(np.float32)).astype(FP8NP)
    return hi, lo


def host_inputs(x_down, x_down_w, a_2, b_2, W, b):
    """Host-side prep: fp8 hi/lo splits, m-major B, prescaled split weights."""
    A2 = np.ascontiguousarray(x_down.reshape(N, L * J).astype(np.float32)) * ASC
    # m-major cols: col = m*L + l
    B2 = np.ascontiguousarray(
        x_down_w.reshape(N, L, J).transpose(0, 2, 1).reshape(N, J * L)
        .astype(np.float32))
    Bh, Bl = _fp8_split(B2)
    # k-interleave: [n, c] -> [n % 128, n // 128, c]
    bh = np.ascontiguousarray(Bh.reshape(2, 128, J * L).transpose(1, 0, 2))
    bl = np.ascontiguousarray(Bl.reshape(2, 128, J * L).transpose(1, 0, 2))

    Wp = a_2.astype(np.float64)[:, None] * W.astype(np.float64)
    s_row = Wp.sum(axis=0)
    # fold the -s[f]*mean[t] LayerNorm correction into the weights
    Wpp = (Wp - s_row[None, :] / D2) * WSC          # [(j*32+m), f], prescaled
    Wh = Wpp.astype(np.float32).astype(FP8NP)
    Wl = (Wpp.astype(np.float32) - Wh.astype(np.float32)).astype(FP8NP)
    Wjmf_h = Wh.reshape(J, J, F)   # [j, m, f]
    Wjmf_l = Wl.reshape(J, J, F)
    # wmain[32g+j, m*F+f] = Wh[j,m,f]; wlow likewise for Wl
    wmain = np.ascontiguousarray(np.tile(Wjmf_h.reshape(J, J * F), (4, 1)))
    wlow = np.ascontiguousarray(np.tile(Wjmf_l.reshape(J, J * F), (4, 1)))

    bconst = b_2.astype(np.float64) @ W.astype(np.float64) + b.astype(np.float64)
    bcol = bconst.astype(np.float32).reshape(F, 1)
    # bones[(32g+j), kt, rt*32 + i'] = 1 where i' == 4*rt + g
    bones = np.zeros((128, 2, NRT * 32), dtype=FP8NP)
    for rt in range(NRT):
        for g in range(4):
            bones[32 * g:32 * (g + 1), :, rt * 32 + 4 * rt + g] = 1.0

    in_maps = []
    for c in range(NCORES):
        Ac = A2[:, c * LI * J:(c + 1) * LI * J]
        Ahc, Alc = _fp8_split(Ac)
        in_maps.append({
            "ah": np.ascontiguousarray(
                Ahc.reshape(2, 128, LI * J).transpose(1, 0, 2)),
            "al": np.ascontiguousarray(
                Alc.reshape(2, 128, LI * J).transpose(1, 0, 2)),
            "bh": bh,
            "bl": bl,
            "wmain": wmain,
            "wlow": wlow,
            "bones": bones,
            "bcol": bcol,
        })
    return in_maps


_NC_CACHE = {}


def _get_program():
    if "nc" not in _NC_CACHE:
        _NC_CACHE["nc"] = build_program()
    return _NC_CACHE["nc"]


def kernel(**inputs) -> np.ndarray:
    nc = _get_program()
    inputs = {k: np.asarray(v) for k, v in inputs.items()}
    in_maps = host_inputs(**inputs)
    trace = bool(int(os.environ.get("COEVOL_TRACE", "0")))
    res = run_bass_kernel_spmd(nc, in_maps, list(range(NCORES)), trace=trace)
    if trace:
        _NC_CACHE["last_result"] = res
    # per-core y is [F, LI*L]; unshard to (B, L, L, F)
    slabs = [res.results[c]["y"].reshape(F, LI, L).transpose(1, 2, 0)
             for c in range(NCORES)]
    return np.concatenate(slabs, axis=0).reshape(B, L, L, F)
